# revision 8
# baseline (speedup 1.0000x reference)
"""HSIC test-statistic kernel for Trainium2, 8-core SPMD.

Row-sharded (n=4096, d=64; 512 rows/core):
  - D tiles come from one augmented PE matmul [-2X|G|1]^T @ [X|1|G] (K=66),
    quantized in PSUM->SBUF to uint16 q = round(clamp(D,0)*128); both q
    matrices stay SBUF-resident (64KB/partition).
  - The off-diagonal median (-> RBF width) is an integer bisection on q:
    exact counts via ACT Sign sweeps with accumulate, one tiny [1,2]
    AllReduce per iteration for the global count (X and Y packed).
  - K = exp(q * -1/(hi-1)) on ACT with accumulated rowsums; centering
    vectors gathered with one AllReduce; testStat/varHSIC sums are fused
    DVE affine_mul_reduce passes over streamed chunks.
  - Host combines 8 partial sums and applies the reference's scalar
    formulas + gamma-quantile bisection in fp32.
"""
import sys

sys.path.insert(0, "/opt/trn_rl_repo")

import numpy as np

N = 4096
D_FEAT = 64
N_CORES = 8
ROWS = N // N_CORES          # 512
RB = ROWS // 128             # 4 row-blocks
QSCALE = 128.0
F = 1024                     # phase-2 column chunk
NCHUNK = N // F              # 4
NS = RB * NCHUNK             # 16 accumulation slots

_CACHE = {}


def _build():
    import concourse.bacc as bacc
    import concourse.tile as tile
    from concourse import mybir

    AF = mybir.ActivationFunctionType
    OP = mybir.AluOpType
    f32 = mybir.dt.float32
    u16 = mybir.dt.uint16
    bf16 = mybir.dt.bfloat16

    nc = bacc.Bacc("TRN2", target_bir_lowering=False, debug=False,
                   enable_asserts=True, num_devices=N_CORES)

    lx_d = nc.dram_tensor("lx", [66, ROWS], f32, kind="ExternalInput").ap()
    ly_d = nc.dram_tensor("ly", [66, ROWS], f32, kind="ExternalInput").ap()
    rx_d = nc.dram_tensor("rx", [66, N], f32, kind="ExternalInput").ap()
    ry_d = nc.dram_tensor("ry", [66, N], f32, kind="ExternalInput").ap()
    sel_d = nc.dram_tensor("sel", [1, N_CORES], f32, kind="ExternalInput").ap()
    out_d = nc.dram_tensor("out", [1, 16], f32, kind="ExternalOutput").ap()

    KTARGET_SIGN = 4096.0    # 2*(8386560+4096) - 4096^2
    LO0, HI0 = 8192.0, 32768.0
    N_ITER = 14

    with tile.TileContext(nc) as tc:
        with tc.tile_pool(name="single", bufs=1) as single, \
             tc.tile_pool(name="sweep", bufs=1) as sweep, \
             tc.tile_pool(name="work", bufs=2) as work, \
             tc.tile_pool(name="psit", bufs=1, space="PSUM") as psit, \
             tc.tile_pool(name="psmm", bufs=3, space="PSUM") as psmm, \
             tc.tile_pool(name="psone", bufs=2, space="PSUM") as psone, \
             tc.tile_pool(name="dram", bufs=1, space="DRAM") as dram:

            ones_col = single.tile([128, 1], f32)
            nc.vector.memset(ones_col[:], 1.0)
            ones_row = single.tile([1, 128], f32)
            nc.vector.memset(ones_row[:], 1.0)

            qx = single.tile([128, RB, N], u16)
            qy = single.tile([128, RB, N], u16)

            # ---------- Phase 0: q = u16(max(D,0)*128), D from augmented matmul
            with tc.tile_pool(name="p0", bufs=1) as p0:
                lx = p0.tile([66, ROWS], f32)
                ly = p0.tile([66, ROWS], f32)
                rx = p0.tile([66, N], f32)
                ry = p0.tile([66, N], f32)
                nc.sync.dma_start(out=lx[:], in_=lx_d[:])
                nc.sync.dma_start(out=ly[:], in_=ly_d[:])
                nc.sync.dma_start(out=rx[:], in_=rx_d[:])
                nc.sync.dma_start(out=ry[:], in_=ry_d[:])
                for (lm, rm, qm) in ((lx, rx, qx), (ly, ry, qy)):
                    for rb in range(RB):
                        for jc in range(N // 512):
                            dp = psmm.tile([128, 512], f32, tag="dp")
                            nc.tensor.matmul(
                                dp[:], lm[:, rb * 128:(rb + 1) * 128],
                                rm[:, jc * 512:(jc + 1) * 512],
                                start=True, stop=True)
                            nc.vector.tensor_scalar(
                                out=qm[:, rb, jc * 512:(jc + 1) * 512],
                                in0=dp[:], scalar1=QSCALE, scalar2=0.0,
                                op0=OP.mult, op1=OP.max)

            # ---------- Phase 1: integer bisection via ACT sign-count sweeps
            lohi = single.tile([1, 4], f32)   # lo_x lo_y hi_x hi_y
            nc.vector.memset(lohi[:, 0:2], LO0)
            nc.vector.memset(lohi[:, 2:4], HI0)
            scr_sgn = sweep.tile([128, RB * N // 2], bf16, tag="scr_sgn")
            sacc4 = single.tile([128, 4], f32)
            ktar_bias = single.tile([1, 1], f32)
            nc.vector.memset(ktar_bias[:], -(KTARGET_SIGN - 0.5))
            agi = dram.tile([1, 2], f32, tag="agi")
            ago = dram.tile([1, 2], f32, tag="ago")

            for it in range(N_ITER):
                mid = work.tile([1, 2], f32, tag="mid")
                nc.vector.tensor_tensor(out=mid[:], in0=lohi[:, 0:2],
                                        in1=lohi[:, 2:4], op=OP.add)
                nc.vector.tensor_scalar(out=mid[:], in0=mid[:], scalar1=0.5,
                                        scalar2=None, op0=OP.mult)
                mb = psit.tile([128, 2], f32, tag="mb")
                nc.tensor.matmul(mb[:], ones_row[:], mid[:], start=True, stop=True)
                midb = work.tile([128, 2], f32, tag="midb")
                nc.vector.tensor_scalar(out=midb[:], in0=mb[:], scalar1=0.5,
                                        scalar2=None, op0=OP.subtract)
                # S = sum sign(mid-0.5-q) = cnt_lt - cnt_ge  (exact, no ties)
                H = RB * N // 2
                qxf = qx[:].rearrange("p r n -> p (r n)")
                qyf = qy[:].rearrange("p r n -> p (r n)")
                for hh in range(2):
                    nc.scalar.activation(out=scr_sgn[:],
                                         in_=qxf[:, hh * H:(hh + 1) * H],
                                         func=AF.Sign, bias=midb[:, 0:1], scale=-1.0,
                                         accum_out=sacc4[:, 0 + hh:1 + hh])
                    nc.scalar.activation(out=scr_sgn[:],
                                         in_=qyf[:, hh * H:(hh + 1) * H],
                                         func=AF.Sign, bias=midb[:, 1:2], scale=-1.0,
                                         accum_out=sacc4[:, 2 + hh:3 + hh])
                sp = psit.tile([1, 4], f32, tag="sp")
                nc.tensor.matmul(sp[:], ones_col[:], sacc4[:], start=True, stop=True)
                sp4 = work.tile([1, 4], f32, tag="sp4")
                nc.vector.tensor_copy(sp4[:], sp[:])
                ssb = work.tile([1, 2], f32, tag="ssb")
                nc.vector.tensor_tensor(out=ssb[:], in0=sp4[:, 0:4:2],
                                        in1=sp4[:, 1:4:2], op=OP.add)
                nc.sync.dma_start(out=agi[:], in_=ssb[:])
                nc.gpsimd.collective_compute(
                    "AllReduce", OP.add,
                    replica_groups=[list(range(N_CORES))],
                    ins=[agi.opt()], outs=[ago.opt()])
                sg = work.tile([1, 2], f32, tag="sg")
                nc.sync.dma_start(out=sg[:], in_=ago[:])
                sgn = work.tile([1, 2], f32, tag="sgn")
                nc.scalar.activation(out=sgn[:], in_=sg[:], func=AF.Sign,
                                     bias=ktar_bias[:], scale=1.0)
                f4 = work.tile([1, 4], f32, tag="f4")
                nc.vector.tensor_scalar(out=f4[:, 2:4], in0=sgn[:], scalar1=0.5,
                                        scalar2=0.5, op0=OP.mult, op1=OP.add)
                nc.vector.tensor_scalar(out=f4[:, 0:2], in0=sgn[:], scalar1=-0.5,
                                        scalar2=0.5, op0=OP.mult, op1=OP.add)
                mid4 = work.tile([1, 4], f32, tag="mid4")
                nc.vector.tensor_copy(mid4[:, 0:2], mid[:])
                nc.vector.tensor_copy(mid4[:, 2:4], mid[:])
                dd = work.tile([1, 4], f32, tag="dd")
                nc.vector.tensor_tensor(out=dd[:], in0=mid4[:], in1=lohi[:],
                                        op=OP.subtract)
                nc.vector.tensor_tensor(out=dd[:], in0=dd[:], in1=f4[:], op=OP.mult)
                nc.vector.tensor_tensor(out=lohi[:], in0=lohi[:], in1=dd[:], op=OP.add)

            # gamma scale = -1/(hi-1), broadcast to [128,2]
            vk = single.tile([1, 2], f32)
            nc.vector.tensor_scalar(out=vk[:], in0=lohi[:, 2:4], scalar1=1.0,
                                    scalar2=None, op0=OP.subtract)
            gsc = single.tile([1, 2], f32)
            nc.vector.reciprocal(gsc[:], vk[:])
            nc.vector.tensor_scalar(out=gsc[:], in0=gsc[:], scalar1=-1.0,
                                    scalar2=None, op0=OP.mult)
            gb = psone.tile([128, 2], f32, tag="oneshot")
            nc.tensor.matmul(gb[:], ones_row[:], gsc[:], start=True, stop=True)
            gscb = single.tile([128, 2], f32)
            nc.vector.tensor_copy(gscb[:], gb[:])

            # ---------- Phase 2a: rowsums of K, L
            rsx = single.tile([128, RB], f32)
            rsy = single.tile([128, RB], f32)
            for (qm, rs, col) in ((qx, rsx, 0), (qy, rsy, 1)):
                for rb in range(RB):
                    scr_exp = sweep.tile([128, N], bf16, tag="scr_exp")
                    nc.scalar.activation(out=scr_exp[:], in_=qm[:, rb, :],
                                         func=AF.Exp, scale=gscb[:, col:col + 1],
                                         accum_out=rs[:, rb:rb + 1])

            # gather rowsums via one-hot zones + AllReduce
            selb = single.tile([1, N_CORES], f32)
            nc.sync.dma_start(out=selb[:], in_=sel_d[:])
            sel128 = psone.tile([128, N_CORES], f32, tag="oneshot")
            nc.tensor.matmul(sel128[:], ones_row[:], selb[:], start=True, stop=True)
            sel128s = single.tile([128, N_CORES], f32)
            nc.vector.tensor_copy(sel128s[:], sel128[:])
            rszx = single.tile([128, N_CORES, RB], f32)
            rszy = single.tile([128, N_CORES, RB], f32)
            for z in range(N_CORES):
                nc.vector.tensor_scalar(out=rszx[:, z, :], in0=rsx[:],
                                        scalar1=sel128s[:, z:z + 1], scalar2=None,
                                        op0=OP.mult)
                nc.vector.tensor_scalar(out=rszy[:, z, :], in0=rsy[:],
                                        scalar1=sel128s[:, z:z + 1], scalar2=None,
                                        op0=OP.mult)
            rs_in = dram.tile([1, 8192], f32, tag="rs_in")
            rs_out = dram.tile([1, 8192], f32, tag="rs_out")
            zpad = sweep.tile([1, 2048], f32, tag="zpad")
            nc.vector.memset(zpad[:], 0.0)
            for zz in range(4):
                nc.sync.dma_start(out=rs_in[:, zz * 2048:(zz + 1) * 2048], in_=zpad[:])
            for z in range(N_CORES):
                for rb in range(RB):
                    o = z * ROWS + rb * 128
                    nc.sync.dma_start(out=rs_in[:, o:o + 128],
                                      in_=rszx[:, z, rb:rb + 1])
                    nc.sync.dma_start(out=rs_in[:, N + o:N + o + 128],
                                      in_=rszy[:, z, rb:rb + 1])
            nc.gpsimd.collective_compute(
                "AllReduce", OP.add,
                replica_groups=[list(range(N_CORES))],
                ins=[rs_in.opt()], outs=[rs_out.opt()])
            # totals: bring rs_out to [128, 64] (p-major) and PE-reduce
            rsg2 = single.tile([128, 64], f32)
            nc.sync.dma_start(out=rsg2[:],
                              in_=rs_out[:, 0:8192].rearrange("o (c p) -> o p c", p=128))
            totp = psone.tile([1, 64], f32, tag="oneshot")
            nc.tensor.matmul(totp[:], ones_col[:], rsg2[:], start=True, stop=True)
            totf = single.tile([1, 64], f32)
            nc.vector.tensor_copy(totf[:], totp[:])
            # fold X cols [0:32], Y cols [32:64] separately
            t16 = single.tile([1, 32], f32)
            nc.vector.tensor_tensor(out=t16[:, 0:16], in0=totf[:, 0:16],
                                    in1=totf[:, 16:32], op=OP.add)
            nc.vector.tensor_tensor(out=t16[:, 16:32], in0=totf[:, 32:48],
                                    in1=totf[:, 48:64], op=OP.add)
            t8 = single.tile([1, 16], f32)
            nc.vector.tensor_tensor(out=t8[:, 0:8], in0=t16[:, 0:8],
                                    in1=t16[:, 8:16], op=OP.add)
            nc.vector.tensor_tensor(out=t8[:, 8:16], in0=t16[:, 16:24],
                                    in1=t16[:, 24:32], op=OP.add)
            t4 = single.tile([1, 8], f32)
            nc.vector.tensor_tensor(out=t4[:, 0:4], in0=t8[:, 0:4],
                                    in1=t8[:, 4:8], op=OP.add)
            nc.vector.tensor_tensor(out=t4[:, 4:8], in0=t8[:, 8:12],
                                    in1=t8[:, 12:16], op=OP.add)
            t2 = single.tile([1, 4], f32)
            nc.vector.tensor_tensor(out=t2[:, 0:2], in0=t4[:, 0:2],
                                    in1=t4[:, 2:4], op=OP.add)
            nc.vector.tensor_tensor(out=t2[:, 2:4], in0=t4[:, 4:6],
                                    in1=t4[:, 6:8], op=OP.add)
            tot2 = single.tile([1, 2], f32)
            nc.vector.tensor_tensor(out=tot2[:, 0:1], in0=t2[:, 0:1],
                                    in1=t2[:, 1:2], op=OP.add)
            nc.vector.tensor_tensor(out=tot2[:, 1:2], in0=t2[:, 2:3],
                                    in1=t2[:, 3:4], op=OP.add)

            tm2 = single.tile([1, 2], f32)
            nc.vector.tensor_scalar(out=tm2[:], in0=tot2[:],
                                    scalar1=1.0 / (N * N), scalar2=None, op0=OP.mult)
            tmb_p = psone.tile([128, 2], f32, tag="oneshot")
            nc.tensor.matmul(tmb_p[:], ones_row[:], tm2[:], start=True, stop=True)
            tmb = single.tile([128, 2], f32)   # tm/2 per matrix
            nc.vector.tensor_scalar(out=tmb[:], in0=tmb_p[:], scalar1=0.5,
                                    scalar2=None, op0=OP.mult)

            # a vectors: a = rs/n - tm/2  (column-broadcast + own-row forms)
            abx = single.tile([128, N], f32)
            aby = single.tile([128, N], f32)
            for (col, ab, off) in ((0, abx, 0), (1, aby, N)):
                rsgh = sweep.tile([1, N], f32, tag="rsgh")
                nc.sync.dma_start(out=rsgh[:], in_=rs_out[:, off:off + N])
                for jc in range(N // 512):
                    bp = psmm.tile([128, 512], f32, tag="dp")
                    nc.tensor.matmul(bp[:], ones_row[:],
                                     rsgh[:, jc * 512:(jc + 1) * 512],
                                     start=True, stop=True)
                    nc.vector.tensor_scalar(out=ab[:, jc * 512:(jc + 1) * 512],
                                            in0=bp[:], scalar1=1.0 / N,
                                            scalar2=tmb[:, col:col + 1],
                                            op0=OP.mult, op1=OP.subtract)
            arx = single.tile([128, RB], f32)
            ary = single.tile([128, RB], f32)
            nc.vector.tensor_scalar(out=arx[:], in0=rsx[:], scalar1=1.0 / N,
                                    scalar2=tmb[:, 0:1], op0=OP.mult, op1=OP.subtract)
            nc.vector.tensor_scalar(out=ary[:], in0=rsy[:], scalar1=1.0 / N,
                                    scalar2=tmb[:, 1:2], op0=OP.mult, op1=OP.subtract)

            # ---------- Phase 2b: streamed S1 = sum Kc*Lc, S2 = sum (Kc*Lc)^2/36
            s1slots = single.tile([128, NS], f32)
            s2slots = single.tile([128, NS], f32)
            for rb in range(RB):
                for ch in range(NCHUNK):
                    kch = work.tile([128, F], f32, tag="kch")
                    lch = work.tile([128, F], f32, tag="lch")
                    nc.scalar.activation(out=kch[:],
                                         in_=qx[:, rb, ch * F:(ch + 1) * F],
                                         func=AF.Exp, scale=gscb[:, 0:1])
                    nc.scalar.activation(out=lch[:],
                                         in_=qy[:, rb, ch * F:(ch + 1) * F],
                                         func=AF.Exp, scale=gscb[:, 1:2])
                    nc.vector.scalar_tensor_tensor(
                        out=kch[:], in0=kch[:], scalar=arx[:, rb:rb + 1],
                        in1=abx[:, ch * F:(ch + 1) * F],
                        op0=OP.subtract, op1=OP.subtract)
                    nc.vector.scalar_tensor_tensor(
                        out=lch[:], in0=lch[:], scalar=ary[:, rb:rb + 1],
                        in1=aby[:, ch * F:(ch + 1) * F],
                        op0=OP.subtract, op1=OP.subtract)
                    m = work.tile([128, F], f32, tag="m")
                    sl = rb * NCHUNK + ch
                    nc.vector.affine_mul_reduce(
                        out=m[:], accum_out=s1slots[:, sl:sl + 1],
                        in0=kch[:], in1=lch[:], scale=1.0, bias=0.0)
                    m2 = work.tile([128, F], f32, tag="kch")
                    nc.vector.affine_mul_reduce(
                        out=m2[:], accum_out=s2slots[:, sl:sl + 1],
                        in0=m[:], in1=m[:], scale=1.0 / 36.0, bias=0.0)

            # trace(V): KcD = 1-2a_i, LcD = 1-2c_i; sum (KcD*LcD)^2/36
            kcd = work.tile([128, RB], f32, tag="kcd")
            nc.vector.tensor_scalar(out=kcd[:], in0=arx[:], scalar1=-2.0,
                                    scalar2=1.0, op0=OP.mult, op1=OP.add)
            lcd = work.tile([128, RB], f32, tag="lcd")
            nc.vector.tensor_scalar(out=lcd[:], in0=ary[:], scalar1=-2.0,
                                    scalar2=1.0, op0=OP.mult, op1=OP.add)
            md = work.tile([128, RB], f32, tag="md")
            nc.vector.tensor_tensor(out=md[:], in0=kcd[:], in1=lcd[:], op=OP.mult)
            mdsq = work.tile([128, RB], f32, tag="mdsq")
            trvacc = single.tile([128, 1], f32)
            nc.vector.affine_mul_reduce(out=mdsq[:], accum_out=trvacc[:],
                                        in0=md[:], in1=md[:],
                                        scale=1.0 / 36.0, bias=0.0)

            # partial sums -> [1,*] and fold
            sp1 = psone.tile([1, NS], f32, tag="oneshot")
            nc.tensor.matmul(sp1[:], ones_col[:], s1slots[:], start=True, stop=True)
            s1f = single.tile([1, NS], f32)
            nc.vector.tensor_copy(s1f[:], sp1[:])
            sp2 = psone.tile([1, NS], f32, tag="oneshot")
            nc.tensor.matmul(sp2[:], ones_col[:], s2slots[:], start=True, stop=True)
            s2f = single.tile([1, NS], f32)
            nc.vector.tensor_copy(s2f[:], sp2[:])
            sp3 = psone.tile([1, 1], f32, tag="oneshot")
            nc.tensor.matmul(sp3[:], ones_col[:], trvacc[:], start=True, stop=True)

            outt = single.tile([1, 16], f32)
            nc.vector.memset(outt[:], 0.0)
            for (src, oidx) in ((s1f, 0), (s2f, 1)):
                a8 = work.tile([1, 8], f32, tag="a8")
                nc.vector.tensor_tensor(out=a8[:], in0=src[:, 0:8],
                                        in1=src[:, 8:16], op=OP.add)
                a4 = work.tile([1, 4], f32, tag="a4")
                nc.vector.tensor_tensor(out=a4[:], in0=a8[:, 0:4],
                                        in1=a8[:, 4:8], op=OP.add)
                a2 = work.tile([1, 2], f32, tag="a2")
                nc.vector.tensor_tensor(out=a2[:], in0=a4[:, 0:2],
                                        in1=a4[:, 2:4], op=OP.add)
                nc.vector.tensor_tensor(out=outt[:, oidx:oidx + 1],
                                        in0=a2[:, 0:1], in1=a2[:, 1:2], op=OP.add)
            nc.vector.tensor_copy(outt[:, 2:3], sp3[:])
            nc.vector.tensor_copy(outt[:, 3:5], tot2[:])
            nc.vector.tensor_copy(outt[:, 5:7], lohi[:, 2:4])
            nc.vector.tensor_copy(outt[:, 7:9], lohi[:, 0:2])
            nc.sync.dma_start(out=out_d[:], in_=outt[:])

    nc.compile()
    return nc


def _get_runner():
    if "runner" in _CACHE:
        return _CACHE["runner"]
    import jax
    from jax.sharding import Mesh, PartitionSpec
    from jax.experimental.shard_map import shard_map
    from concourse import mybir
    from concourse.bass2jax import (_bass_exec_p, install_neuronx_cc_hook,
                                    partition_id_tensor)
    nc = _build()
    install_neuronx_cc_hook()
    partition_name = nc.partition_id_tensor.name if nc.partition_id_tensor else None
    in_names, out_names, out_avals, zero_outs = [], [], [], []
    for alloc in nc.m.functions[0].allocations:
        if not isinstance(alloc, mybir.MemoryLocationSet):
            continue
        name = alloc.memorylocations[0].name
        if alloc.kind == "ExternalInput":
            if name != partition_name:
                in_names.append(name)
        elif alloc.kind == "ExternalOutput":
            shape = tuple(alloc.tensor_shape)
            dtype = mybir.dt.np(alloc.dtype)
            out_names.append(name)
            out_avals.append(jax.core.ShapedArray(shape, dtype))
            zero_outs.append(np.zeros(shape, dtype))
    n_params = len(in_names)
    all_in_names = list(in_names) + list(out_names)
    if partition_name is not None:
        all_in_names.append(partition_name)

    def _body(*args):
        operands = list(args)
        if partition_name is not None:
            operands.append(partition_id_tensor())
        outs = _bass_exec_p.bind(
            *operands, out_avals=tuple(out_avals), in_names=tuple(all_in_names),
            out_names=tuple(out_names), lowering_input_output_aliases=(),
            sim_require_finite=True, sim_require_nnan=True, nc=nc)
        return tuple(outs)

    devices = jax.devices()[:N_CORES]
    mesh = Mesh(np.asarray(devices), ("core",))
    n_outs = len(out_avals)
    sharded = jax.jit(
        shard_map(_body, mesh=mesh,
                  in_specs=(PartitionSpec("core"),) * (n_params + n_outs),
                  out_specs=(PartitionSpec("core"),) * n_outs, check_rep=False),
        keep_unused=True)

    def run(in_maps):
        per_core = [[np.asarray(m[name]) for name in in_names] for m in in_maps]
        concat_in = [np.concatenate([per_core[c][i] for c in range(N_CORES)], axis=0)
                     for i in range(n_params)]
        concat_zeros = [np.zeros((N_CORES * z.shape[0], *z.shape[1:]), z.dtype)
                        for z in zero_outs]
        out_arrs = sharded(*concat_in, *concat_zeros)
        return [
            {name: np.asarray(out_arrs[i]).reshape(N_CORES, *out_avals[i].shape)[c]
             for i, name in enumerate(out_names)}
            for c in range(N_CORES)
        ]

    _CACHE["runner"] = (run, nc)
    return _CACHE["runner"]


def _gamma_ppf_f32(a, p):
    """Mirror reference._gamma_ppf: 100-iteration bisection in fp32."""
    try:
        from scipy.special import gammainc as _ginc

        def ginc(a_, x_):
            return np.float32(_ginc(np.float64(a_), np.float64(x_)))
    except ImportError:
        import jax

        with jax.default_device(jax.devices("cpu")[0]):
            from jax.scipy.special import gammainc as _jginc

            def ginc(a_, x_):
                return np.float32(_jginc(np.float32(a_), np.float32(x_)))
    a = np.float32(a)
    p = np.float32(p)
    lo = np.float32(0.0)
    hi = np.float32(np.float32(a + np.float32(10.0) * np.sqrt(a)) + np.float32(100.0))
    for _ in range(100):
        mid = np.float32(0.5) * (lo + hi)
        if ginc(a, mid) < p:
            lo = mid
        else:
            hi = mid
    return np.float32(0.5) * (lo + hi)


def kernel(X, Y):
    X = np.asarray(X, dtype=np.float32)
    Y = np.asarray(Y, dtype=np.float32)
    n = X.shape[0]
    assert n == N and X.shape[1] == D_FEAT

    run, _nc = _get_runner()

    def prep(M):
        Mt = np.ascontiguousarray(M.T)
        G = (M ** 2).sum(axis=1).astype(np.float32)
        R = np.concatenate([Mt, np.ones((1, N), np.float32), G[None, :]], axis=0)
        Ls = []
        for c in range(N_CORES):
            sl = slice(c * ROWS, (c + 1) * ROWS)
            L = np.concatenate([-2.0 * Mt[:, sl], G[None, sl],
                                np.ones((1, ROWS), np.float32)], axis=0)
            Ls.append(np.ascontiguousarray(L))
        return np.ascontiguousarray(R), Ls

    RX, LXs = prep(X)
    RY, LYs = prep(Y)
    in_maps = []
    for c in range(N_CORES):
        sel = np.zeros((1, N_CORES), np.float32)
        sel[0, c] = 1.0
        in_maps.append({"lx": LXs[c], "ly": LYs[c], "rx": RX, "ry": RY, "sel": sel})

    results = run(in_maps)

    outs = np.stack([r["out"][0] for r in results])  # [8, 16]
    S1 = np.float32(outs[:, 0].sum(dtype=np.float64))
    S2 = np.float32(outs[:, 1].sum(dtype=np.float64))
    trV = np.float32(outs[:, 2].sum(dtype=np.float64))
    totX = np.float32(outs[0, 3])
    totY = np.float32(outs[0, 4])

    nf = np.float32(n)
    testStat = S1 / nf
    varHSIC = (S2 - trV) / nf / np.float32(n - 1)
    varHSIC = varHSIC * np.float32(72.0) * np.float32(n - 4) * np.float32(n - 5) \
        / nf / np.float32(n - 1) / np.float32(n - 2) / np.float32(n - 3)
    K0sum = totX - nf
    L0sum = totY - nf
    muX = K0sum / nf / np.float32(n - 1)
    muY = L0sum / nf / np.float32(n - 1)
    mHSIC = (np.float32(1.0) + muX * muY - muX - muY) / nf
    al = mHSIC ** 2 / varHSIC
    bet = varHSIC * nf / mHSIC
    thresh = bet * _gamma_ppf_f32(al, np.float32(0.2))
    return (np.float32(testStat), np.float32(thresh))
